# revision 51
# baseline (speedup 1.0000x reference)
"""Trainium2 (8 NeuronCores) Bass kernel for nn_AdaptiveInteraction.

Math (per sample b, N=3000, D=64):
    Ein  = input @ W^T + b1                      [N, D]
    S    = Ein Ein^T / sqrt(D)                   [N, N]
    E    = S Ein                                 [N, D]
    BatchNorm over (B,N):  Ehat = g*(E-mu)*rsqrt(var+eps) + beta
    A    = softmax(relu(Ehat E^T), axis=-1)      [N, N]
    out[k,b,i,j] = m[k,j] * A[b,i,j]             [K,B,N,N]

Key algebra: with Xa = [x | 1] (augmented), Wa = [[W^T; b1] | e64],
G_aug = Wa^T (Xa^T Xa) Wa, Gs = G_aug[0:64,:]/8, the whole pre-softmax
reduces to 64x64 products and the logits become a single rank-65 product
against the transposed raw input:
    logits = v2^T xt,  v2 = Wa64 (Mq Ein_r^T + u 1^T)  [65, 750]
    Mq = Gs64 diag(gp) Gs64,  u = Gs64 cneg,
    gp = gamma*rsqrt(var+eps), cneg = beta - gp*mu
so no NxN intermediate and no einT tensor at all; xt (the transposed
input) is built on-device with PE transposes from the natural-layout
chunks.  BN stats come from Gs of both samples (computed locally on
every core; no collectives).

Phase 5 per 125-row chunk: PE writes logits into three [128,1024]
PSUM pieces; per-piece local-max softmax (DVE reduce_max from PSUM,
Act exp with bf16 output and accumulated rowsums; the global max
correction e_p and 1/S fold into per-piece q-multiplies split between
Act scalar.mul and a 4x DVE tensor_scalar).  The two m-row weightings
are 2x bf16 tensor_tensors split DVE (o0) / Pool (most of o1), and
both outputs leave in one k-interleaved bf16 DMA per chunk (half the
write traffic); the host upcasts to f32 while unsharding.

The actual v2 build differs slightly from the sketch above: with
Q = Gs Ein_r^T the fused form v2 = (Gs Wa^T)^T (gp (x) Q + cneg (x) 1)
needs only one tensor_scalar plus one small matmul after the BN stats
land, keeping the stats->logits critical path to a handful of ops.

Sharding: 8 cores = (B=2 samples) x (4 row-blocks of 750 rows).
"""

import sys

for _p in ("/opt/trn_rl_repo", "/root/.axon_site/_ro/trn_rl_repo"):
    if _p not in sys.path:
        sys.path.insert(0, _p)

import numpy as np

B, N, DIN, D, K = 2, 3000, 64, 64, 2
NP = 3072          # padded j dimension (24 * 128)
R = 750            # rows per core
IC = 125           # rows per i-chunk (6 chunks per core)
NCH = 6
EPS = 1e-5
NCORES = 8
NCHK = NP // 128   # 24 j-chunks per sample

# phase-5 column pieces (global col base, width); psum tiles are 1024 wide
PIECES = [(0, 1024), (1024, 1024), (2048, 952)]

_CACHE = {}


def build_nc():
    import concourse.mybir as mybir
    from concourse import bacc
    from concourse.tile import TileContext

    f32 = mybir.dt.float32
    f32r = mybir.dt.float32r
    bf16 = mybir.dt.bfloat16
    u32 = mybir.dt.uint32
    Alu = mybir.AluOpType
    Act = mybir.ActivationFunctionType
    AX = mybir.AxisListType

    nc = bacc.Bacc(num_devices=NCORES)

    # natural-layout augmented x, pre-chunked host-side to [128, 24*65]
    xnm = nc.declare_dram_parameter("xnm", [128, NCHK * (DIN + 1)], f32, isOutput=False)
    xno = nc.declare_dram_parameter("xno", [128, NCHK * (DIN + 1)], f32, isOutput=False)
    # transposed augmented x for this core's row block
    xtr = nc.declare_dram_parameter("xtr", [DIN + 1, R], f32, isOutput=False)
    # packed weights: cols 0:65 Wa, 66:131 Wa^T (rows 0:64, col 131 zero
    # pad so f32r matmul free dims stay even), 133 gamma, 134 beta
    wgb = nc.declare_dram_parameter("wgb", [DIN + 1, 136], f32, isOutput=False)
    m0_p = nc.declare_dram_parameter("m0b", [1, 3008], bf16, isOutput=False)
    m1_p = nc.declare_dram_parameter("m1b", [1, 3008], bf16, isOutput=False)
    id_p = nc.declare_dram_parameter("ident", [128, 128], f32, isOutput=False)
    out_p = nc.declare_dram_parameter("out", [R, K, N], bf16, isOutput=True)

    with TileContext(nc, num_cores=NCORES) as tc:
        with tc.tile_pool(name="const", bufs=1) as cp:
            xn_m = cp.tile([128, NCHK, DIN + 1], f32)
            xn_o = cp.tile([128, NCHK, DIN + 1], f32)
            xtr_sb = cp.tile([DIN + 1, R], f32)
            wgb_sb = cp.tile([DIN + 1, 136], f32)
            ident = cp.tile([128, 128], f32)
            wat_r = cp.tile([D, DIN + 2], f32r)
            xt_sb = cp.tile([DIN + 1, NP], f32r)
            m0b = cp.tile([1, 3008], bf16)
            m1b = cp.tile([1, 3008], bf16)
            mb0 = cp.tile([128, 3008], bf16)
            mb1 = cp.tile([128, 3008], bf16)
            gs_m = cp.tile([D, D + 1], f32r)
            gs_o = cp.tile([D, D + 1], f32r)
            ert_sb = cp.tile([D, R], f32r)
            q_sb = cp.tile([D, R], f32r)
            gpqc = cp.tile([D, 768], f32r)
            w3t_sb = cp.tile([D, D + 2], f32r)
            v2_sb = cp.tile([DIN + 1, 832], f32r)
            sm = cp.tile([128, 16], f32)
            sq = cp.tile([D, 2 * D], f32)
            warm = cp.tile([DIN + 1, 512], bf16)

            # ---- loads: spread issue over SP / Act / Pool queues so the
            # critical xn transfers hit the DMA engines early ----
            HC = (NCHK // 2) * (DIN + 1)
            QC = (NCHK // 4) * (DIN + 1)
            nc.sync.dma_start(
                out=xn_m[:, 0 : NCHK // 4, :].rearrange("p c d -> p (c d)"),
                in_=xnm[:, 0:QC],
            )
            nc.sync.dma_start(out=ident[:, :], in_=id_p[:, :])
            for qq in range(1, 4):
                nc.sync.dma_start(
                    out=xn_m[
                        :, qq * (NCHK // 4) : (qq + 1) * (NCHK // 4), :
                    ].rearrange("p c d -> p (c d)"),
                    in_=xnm[:, qq * QC : (qq + 1) * QC],
                )
            nc.sync.dma_start(out=xtr_sb[:, :], in_=xtr[:, :])
            nc.sync.dma_start(out=wgb_sb[:, :], in_=wgb[:, :])
            nc.scalar.dma_start(
                out=xn_o[:, : NCHK // 2, :].rearrange("p c d -> p (c d)"),
                in_=xno[:, 0:HC],
            )
            nc.scalar.dma_start(
                out=xn_o[:, NCHK // 2 :, :].rearrange("p c d -> p (c d)"),
                in_=xno[:, HC:],
            )
            nc.sync.dma_start(out=m0b[:, :], in_=m0_p[:, :])
            nc.sync.dma_start(out=m1b[:, :], in_=m1_p[:, :])

            # small const prep
            nc.vector.memset(warm[:, :], 0.0)
            nc.vector.memset(gpqc[:, :].bitcast(u32), 0)
            nc.vector.memset(v2_sb[:, :].bitcast(u32), 0)
            nc.vector.memset(sm[:, 11:12].bitcast(u32), 0x5F3759DF)
            nc.vector.memset(sm[:, 14:15].bitcast(u32), 0)
            nc.scalar.activation(sm[0:1, 15:16], sm[0:1, 14:15], Act.Exp)
            nc.vector.tensor_copy(wat_r[:, :], wgb_sb[0:D, 66:132])


            nc.gpsimd.partition_broadcast(mb0[:, :], m0b[:, :])
            nc.gpsimd.partition_broadcast(mb1[:, :], m1b[:, :])

            g_col = wgb_sb[0:D, 133:134]
            bt_col = wgb_sb[0:D, 134:135]

            # ---- pool A: warm PE, G both samples (interleaved halves),
            # transposes, per-sample stats, gs_m-dependent prep (ert/Q/ET)
            with tc.tile_pool(name="psA", bufs=1, space="PSUM") as pa:
                warm_ps = pa.tile([DIN + 1, 512], f32, tag="xtg", name="warmps", bufs=2)

                def warm_pe(n):
                    for _ in range(n):
                        nc.tensor.matmul(
                            warm_ps[:, :], lhsT=warm[:, 0 : DIN + 1],
                            rhs=warm[:, :], start=True, stop=True,
                        )

                warm_pe(8)  # span the xn load window, ramp to full pstate

                xtg = [None] * 6
                xxp = {}

                def g_half(xsrc, tagc, h, transpose):
                    if tagc not in xxp:
                        xxp[tagc] = pa.tile(
                            [DIN + 1, DIN + 1], f32, tag="xx", name=f"xx{tagc}", bufs=3
                        )
                    for c in range(12 * h, 12 * h + 12):
                        nc.tensor.matmul(
                            xxp[tagc][:, :], lhsT=xsrc[:, c, :], rhs=xsrc[:, c, :],
                            start=(c == 0), stop=(c == NCHK - 1),
                        )
                        if transpose:
                            g = c // 4
                            if xtg[g] is None:
                                xtg[g] = pa.tile(
                                    [DIN + 1, 512], f32, tag="xtg", name=f"xtg{g}", bufs=2
                                )
                            nc.tensor.transpose(
                                xtg[g][:, 128 * (c % 4) : 128 * (c % 4 + 1)],
                                xsrc[:, c, :],
                                ident[:, :],
                            )
                            if c % 4 == 3:
                                if g % 2 == 0:
                                    nc.vector.tensor_copy(
                                        xt_sb[:, 512 * g : 512 * (g + 1)], xtg[g][:, :]
                                    )
                                else:
                                    nc.scalar.copy(
                                        xt_sb[:, 512 * g : 512 * (g + 1)], xtg[g][:, :]
                                    )

                def g_tail(gdst, tagc):
                    xx_sb = cp.tile([DIN + 1, DIN + 1], f32, name=f"xxsb{tagc}")
                    nc.vector.tensor_copy(xx_sb[:, :], xxp[tagc][:, :])
                    s2p = pa.tile([DIN + 1, DIN + 1], f32, tag="xx", name=f"s2{tagc}", bufs=3)
                    nc.tensor.matmul(
                        s2p[:, :], lhsT=xx_sb[:, :], rhs=wgb_sb[:, 0 : DIN + 1],
                        start=True, stop=True,
                    )
                    s2_sb = cp.tile([DIN + 1, DIN + 1], f32, name=f"s2sb{tagc}")
                    nc.vector.tensor_copy(s2_sb[:, :], s2p[:, :])
                    gap = pa.tile([DIN + 1, DIN + 1], f32, tag="xx", name=f"ga{tagc}", bufs=3)
                    nc.tensor.matmul(
                        gap[:, :], lhsT=wgb_sb[:, 0 : DIN + 1], rhs=s2_sb[:, :],
                        start=True, stop=True,
                    )
                    nc.vector.tensor_scalar_mul(gdst[:, :], gap[0:D, 0 : D + 1], 0.125)

                def sample_stats(gsx, smp):
                    # q8 = Gs^T Gs in cols 0:64, s1 (colsum(E)/8) in cols 64:66
                    qs1 = pa.tile([DIN + 1, 68], f32, tag="q8s", name=f"q8s{smp}", bufs=1)
                    nc.tensor.matmul(
                        qs1[0:D, 0:D], lhsT=gsx[:, 0:D], rhs=gsx[:, 0:D],
                        start=True, stop=True,
                    )
                    nc.tensor.matmul(
                        qs1[0:D, D : D + 2], lhsT=gsx[:, 0:D], rhs=gsx[:, D - 1 : D + 1],
                        start=True, stop=True,
                    )
                    nc.vector.tensor_tensor(
                        sq[:, D * smp : D * (smp + 1)], qs1[0:D, 0:D], gsx[:, 0:D], Alu.mult
                    )
                    nc.vector.reduce_sum(
                        sm[0:D, smp : smp + 1], sq[:, D * smp : D * (smp + 1)], axis=AX.X
                    )
                    nc.vector.tensor_copy(sm[0:D, 2 + smp : 3 + smp], qs1[0:D, D + 1 : D + 2])

                g_half(xn_m, "m", 0, True)
                g_half(xn_o, "o", 0, False)
                g_half(xn_m, "m", 1, True)
                g_half(xn_o, "o", 1, False)
                g_tail(gs_m, "m")
                sample_stats(gs_m, 0)
                # ert = Ein_r^T (needs only wgb/xtr); its Act copy overlaps
                # the other sample's tail below
                eqp = pa.tile([D, 768], f32, tag="eq", name="ertps", bufs=1)
                for c0, c1 in ((0, 512), (512, R)):
                    nc.tensor.matmul(
                        eqp[:, c0:c1], lhsT=wgb_sb[:, 0:D], rhs=xtr_sb[:, c0:c1],
                        start=True, stop=True,
                    )
                nc.scalar.copy(ert_sb[:, :], eqp[:, 0:R])
                g_tail(gs_o, "o")
                sample_stats(gs_o, 1)
                # Q = Gs ert;  W3T = Gs Wa^T (v2 lhsT, absorbs the Gs)
                qp = pa.tile([D, 768], f32, tag="eq", name="qps", bufs=1)
                for c0, c1 in ((0, 512), (512, R)):
                    nc.tensor.matmul(
                        qp[:, c0:c1], lhsT=gs_m[:, 0:D], rhs=ert_sb[:, c0:c1],
                        start=True, stop=True,
                    )
                nc.scalar.copy(q_sb[:, :], qp[:, 0:R])
                w3p = pa.tile([DIN + 1, 68], f32, tag="q8s", name="w3ps", bufs=1)
                nc.tensor.matmul(
                    w3p[0:D, 0 : D + 2], lhsT=gs_m[:, 0:D], rhs=wat_r[:, :],
                    start=True, stop=True,
                )
                nc.vector.tensor_copy(w3t_sb[:, :], w3p[0:D, 0 : D + 2])

            # ---- stats combine (fused tensor_scalar ops, short chain) ----
            mean = sm[0:D, 4:5]
            ex2 = sm[0:D, 5:6]
            msq = sm[0:D, 6:7]
            vpe = sm[0:D, 7:8]
            rstd = sm[0:D, 8:9]
            gp = sm[0:D, 9:10]
            negc = sm[0:D, 10:11]
            magic = sm[0:D, 11:12]
            i2 = sm[0:D, 12:13]
            t1 = sm[0:D, 13:14]
            cnt8 = 8.0 / float(B * N)
            nc.vector.tensor_scalar(
                mean, sm[0:D, 2:3], sm[0:D, 3:4], cnt8, Alu.add, Alu.mult
            )
            nc.vector.tensor_scalar(
                ex2, sm[0:D, 0:1], sm[0:D, 1:2], cnt8, Alu.add, Alu.mult
            )
            nc.vector.tensor_tensor(msq, mean, mean, Alu.mult)
            nc.vector.tensor_scalar(
                vpe, ex2, msq, float(EPS), Alu.subtract, Alu.add
            )
            # rstd = (var+eps)^-0.5: fast-inverse-sqrt seed + 2 Newton
            # steps (DVE-only: Act Sqrt would thrash the act-func table)
            nc.vector.tensor_scalar(
                i2.bitcast(mybir.dt.int32), vpe.bitcast(mybir.dt.int32),
                1, None, Alu.arith_shift_right,
            )
            nc.vector.tensor_tensor(
                rstd.bitcast(mybir.dt.int32), magic.bitcast(mybir.dt.int32),
                i2.bitcast(mybir.dt.int32), Alu.subtract,
            )
            for _ in range(2):
                nc.vector.tensor_tensor(t1, vpe, rstd, Alu.mult)
                nc.vector.tensor_tensor(t1, t1, rstd, Alu.mult)
                nc.vector.tensor_scalar(t1, t1, -0.5, 1.5, Alu.mult, Alu.add)
                nc.vector.tensor_tensor(rstd, rstd, t1, Alu.mult)
            nc.vector.tensor_tensor(gp, g_col, rstd, Alu.mult)
            nc.vector.tensor_scalar(
                negc, gp, mean, bt_col, Alu.mult, Alu.subtract
            )
            # phase-5 lhsT: v2 = Wa64 (gp (x) Q + cneg (x) ones)
            nc.vector.tensor_scalar(
                gpqc[:, 0:R], q_sb[:, :], gp, negc, Alu.mult, Alu.subtract
            )
            with tc.tile_pool(name="psB", bufs=1, space="PSUM") as pb:
                v2ps = pb.tile([DIN + 1, 768], f32, name="v2ps")
                for c0, c1 in ((0, 512), (512, R)):
                    nc.tensor.matmul(
                        v2ps[:, c0:c1], lhsT=w3t_sb[:, 0 : DIN + 1], rhs=gpqc[:, c0:c1],
                        start=True, stop=True,
                    )
                nc.scalar.copy(v2_sb[:, 0:R], v2ps[:, 0:R])

            # ---- phase 5: logits, softmax, weighted bf16 outputs ----
            # 3 psum pieces (1024/1024/952) cover the 3000 cols + pad.
            # Per-piece local-max softmax: exp(piece) only waits its own row
            # max; the global correction e_p = exp(m_p - M) and 1/S fold
            # into per-piece q-multiplies (g_p), split Act/DVE.  o0 = q*m0
            # on DVE, o1 = q*m1 mostly on Pool; one k-interleaved DMA/chunk.
            OSP = 2880   # Pool's share of the o1 weighting
            W08 = 3008
            PIECES5 = ((0, 1024), (1024, 1024), (2048, 960))
            with (
                tc.tile_pool(name="psL", bufs=1, space="PSUM") as pl,
                tc.tile_pool(name="pex", bufs=5) as pex,
                tc.tile_pool(name="pq", bufs=5) as pq,
                tc.tile_pool(name="pout", bufs=4) as pout,
                tc.tile_pool(name="pnm", bufs=4) as pnm,
            ):
                st = [dict() for _ in range(NCH)]

                def s1(c):
                    lgs = []
                    for p, (base, w) in enumerate(PIECES5):
                        lg = pl.tile([128, 1024], f32, tag="lg", name=f"lg{c}_{p}", bufs=4)
                        for c0, c1 in ((0, 512), (512, w)):
                            nc.tensor.matmul(
                                lg[:, c0:c1],
                                lhsT=v2_sb[:, IC * c : IC * c + 128],
                                rhs=xt_sb[:, base + c0 : base + c1],
                                start=True, stop=True,
                            )
                        lgs.append(lg)
                    st[c]["lg"] = lgs

                def s2(c):
                    # nm cols: 0:3 -localmax_p, 3 negM (clamped), 4:7 S_p,
                    # 7 S, 8 invS, 9:12 e_p -> g_p
                    nm = pnm.tile([128, 16], f32, tag="nm", name=f"nm{c}")
                    lgs = st[c]["lg"]
                    for p, (base, w) in enumerate(PIECES5):
                        nc.vector.reduce_max(
                            nm[:, p : p + 1], lgs[p][:, 0:w], axis=AX.X, negate=True
                        )
                    nc.vector.tensor_reduce(
                        nm[:, 3:4], nm[:, 0:3], axis=AX.X, op=Alu.min
                    )
                    nc.vector.tensor_scalar_min(nm[:, 3:4], nm[:, 3:4], 0.0)
                    st[c]["nm"] = nm

                def s3(c):
                    nm = st[c]["nm"]
                    lgs = st[c]["lg"]
                    pexp = pex.tile([128, W08], bf16, tag="pex", name=f"pex{c}")
                    for p, (base, w) in enumerate(PIECES5):
                        we = min(w, W08 - base)
                        nc.scalar.activation(
                            pexp[:, base : base + we],
                            lgs[p][:, 0:we],
                            Act.Exp,
                            bias=nm[:, p : p + 1],
                            accum_out=nm[:, 4 + p : 5 + p],
                        )
                    nc.scalar.activation(
                        nm[:, 9:12], nm[:, 0:3], Act.Exp,
                        bias=nm[:, 3:4], scale=-1.0,
                    )
                    nc.vector.tensor_tensor(
                        nm[:, 4:7], nm[:, 4:7], nm[:, 9:12], Alu.mult
                    )
                    nc.vector.reduce_sum(nm[:, 7:8], nm[:, 4:7], axis=AX.X)
                    nc.vector.reciprocal(nm[:, 8:9], nm[:, 7:8])
                    nc.vector.tensor_scalar(
                        nm[:, 9:12], nm[:, 9:12], nm[:, 8:9], None, Alu.mult
                    )
                    st[c]["p"] = pexp

                def s4(c):
                    r0 = IC * c
                    nm = st[c]["nm"]
                    pexp = st[c]["p"]
                    last = c == NCH - 1
                    q = pq.tile([128, W08], bf16, tag="q", name=f"q{c}")
                    if c == 0 or last:
                        nc.vector.tensor_scalar(
                            q[:, 0:1024], pexp[:, 0:1024], nm[:, 9:10], None, Alu.mult
                        )
                        nc.vector.tensor_scalar(
                            q[:, 1024:2048], pexp[:, 1024:2048], nm[:, 10:11],
                            None, Alu.mult,
                        )
                    else:
                        nc.scalar.mul(q[:, 0:1024], pexp[:, 0:1024], nm[:, 9:10])
                        nc.scalar.mul(q[:, 1024:2048], pexp[:, 1024:2048], nm[:, 10:11])
                    nc.vector.tensor_scalar(
                        q[:, 2048:W08], pexp[:, 2048:W08], nm[:, 11:12], None, Alu.mult
                    )
                    o01 = pout.tile([128, 2, W08], bf16, tag="o", name=f"o{c}")
                    osp = 1600 if last else OSP
                    nc.gpsimd.tensor_tensor(
                        o01[:, 1, 0:osp], q[:, 0:osp], mb1[:, 0:osp], Alu.mult
                    )
                    nc.vector.tensor_tensor(
                        o01[:, 0, :], q[:, :], mb0[:, :], Alu.mult
                    )
                    if last:
                        # drain: let the o0 half leave while Pool finishes o1
                        nc.sync.dma_start(
                            out=out_p[r0 : r0 + IC, 0:1, :],
                            in_=o01[0:IC, 0:1, 0:N],
                        )
                    nc.vector.tensor_tensor(
                        o01[:, 1, osp:W08], q[:, osp:W08], mb1[:, osp:W08], Alu.mult
                    )
                    if last:
                        nc.sync.dma_start(
                            out=out_p[r0 : r0 + IC, 1:2, :],
                            in_=o01[0:IC, 1:2, 0:N],
                        )
                    else:
                        nc.sync.dma_start(
                            out=out_p[r0 : r0 + IC, :, :], in_=o01[0:IC, :, 0:N]
                        )

                stages = (s1, s2, s3, s4)
                for step in range(NCH + 3):
                    for k, fn in enumerate(stages):
                        c = step - k
                        if 0 <= c < NCH:
                            fn(c)

    nc.compile()
    return nc


def make_in_maps(inputs):
    inp = np.asarray(inputs["input"], dtype=np.float32)
    m = np.asarray(inputs["m"], dtype=np.float32)
    W = np.asarray(inputs["W_in1"], dtype=np.float32)
    b1 = np.asarray(inputs["b_in1"], dtype=np.float32)
    g = np.asarray(inputs["bn2_gamma"], dtype=np.float32)
    bt = np.asarray(inputs["bn2_beta"], dtype=np.float32)

    wa = np.zeros((DIN + 1, D + 1), dtype=np.float32)
    wa[:DIN, :D] = W.T
    wa[DIN, :D] = b1
    wa[DIN, D] = 1.0  # unit column: passes the x ones-row through
    wgb = np.zeros((DIN + 1, 136), dtype=np.float32)
    wgb[:, 0 : D + 1] = wa
    wgb[0:D, 66:131] = wa.T[:D, :]
    wgb[0:D, 133] = g
    wgb[0:D, 134] = bt
    wgb = np.ascontiguousarray(wgb)
    import ml_dtypes
    bf = ml_dtypes.bfloat16
    mpad = np.zeros((K, 3008), dtype=np.float32)
    mpad[:, :N] = m
    m0b = np.ascontiguousarray(mpad[0:1, :].astype(bf))
    m1b = np.ascontiguousarray(mpad[1:2, :].astype(bf))
    ident = np.ascontiguousarray(np.eye(128, dtype=np.float32))

    xts = []
    xns = []
    for b in range(B):
        x = np.zeros((DIN + 1, NP), dtype=np.float32)
        x[:DIN, :N] = inp[b].T
        x[DIN, :N] = 1.0  # ones row (zero on the j-padding)
        xts.append(x)
        # natural layout, pre-chunked to [128, 24*(D+1)] for straight DMA
        xn = np.ascontiguousarray(
            x.T.reshape(NP // 128, 128, DIN + 1)
            .transpose(1, 0, 2)
            .reshape(128, (NP // 128) * (DIN + 1))
        )
        xns.append(xn)

    in_maps = []
    for c in range(NCORES):
        b, r = divmod(c, 4)
        in_maps.append(
            {
                "xnm": xns[b],
                "xno": xns[1 - b],
                "xtr": np.ascontiguousarray(xts[b][:, R * r : R * (r + 1)]),
                "wgb": wgb,
                "m0b": m0b,
                "m1b": m1b,
                "ident": ident,
            }
        )
    return in_maps


def kernel(**inputs):
    from concourse.bass_utils import run_bass_kernel_spmd

    if "nc" not in _CACHE:
        _CACHE["nc"] = build_nc()
    nc = _CACHE["nc"]
    in_maps = make_in_maps(inputs)
    res = run_bass_kernel_spmd(nc, in_maps, core_ids=list(range(NCORES))).results

    out = np.empty((K, B, N, N), dtype=np.float32)
    for c in range(NCORES):
        b, r = divmod(c, 4)
        out[:, b, R * r : R * (r + 1), :] = (
            np.asarray(res[c]["out"]).astype(np.float32).transpose(1, 0, 2)
        )
    return out


# revision 53
# speedup vs baseline: 1.0091x; 1.0091x over previous
"""Trainium2 (8 NeuronCores) Bass kernel for nn_AdaptiveInteraction.

Math (per sample b, N=3000, D=64):
    Ein  = input @ W^T + b1                      [N, D]
    S    = Ein Ein^T / sqrt(D)                   [N, N]
    E    = S Ein                                 [N, D]
    BatchNorm over (B,N):  Ehat = g*(E-mu)*rsqrt(var+eps) + beta
    A    = softmax(relu(Ehat E^T), axis=-1)      [N, N]
    out[k,b,i,j] = m[k,j] * A[b,i,j]             [K,B,N,N]

Key algebra: with Xa = [x | 1] (augmented), Wa = [[W^T; b1] | e64],
G_aug = Wa^T (Xa^T Xa) Wa, Gs = G_aug[0:64,:]/8, the whole pre-softmax
reduces to 64x64 products and the logits become a single rank-65 product
against the transposed raw input:
    logits = v2^T xt,  v2 = Wa64 (Mq Ein_r^T + u 1^T)  [65, 750]
    Mq = Gs64 diag(gp) Gs64,  u = Gs64 cneg,
    gp = gamma*rsqrt(var+eps), cneg = beta - gp*mu
so no NxN intermediate and no einT tensor at all; xt (the transposed
input) is built on-device with PE transposes from the natural-layout
chunks.  BN stats come from Gs of both samples (computed locally on
every core; no collectives).

Phase 5 per 125-row chunk: PE writes logits into three [128,1024]
PSUM pieces; per-piece local-max softmax (DVE reduce_max from PSUM,
Act exp with bf16 output and accumulated rowsums; the global max
correction e_p and 1/S fold into per-piece q-multiplies split between
Act scalar.mul and a 4x DVE tensor_scalar).  The two m-row weightings
are 2x bf16 tensor_tensors split DVE (o0) / Pool (most of o1), and
both outputs leave in one k-interleaved bf16 DMA per chunk (half the
write traffic); the host upcasts to f32 while unsharding.

The actual v2 build differs slightly from the sketch above: with
Q = Gs Ein_r^T the fused form v2 = (Gs Wa^T)^T (gp (x) Q + cneg (x) 1)
needs only one tensor_scalar plus one small matmul after the BN stats
land, keeping the stats->logits critical path to a handful of ops.

Sharding: 8 cores = (B=2 samples) x (4 row-blocks of 750 rows).
"""

import sys

for _p in ("/opt/trn_rl_repo", "/root/.axon_site/_ro/trn_rl_repo"):
    if _p not in sys.path:
        sys.path.insert(0, _p)

import numpy as np

B, N, DIN, D, K = 2, 3000, 64, 64, 2
NP = 3072          # padded j dimension (24 * 128)
R = 750            # rows per core
IC = 125           # rows per i-chunk (6 chunks per core)
NCH = 6
EPS = 1e-5
NCORES = 8
NCHK = NP // 128   # 24 j-chunks per sample

# phase-5 column pieces (global col base, width); psum tiles are 1024 wide
PIECES = [(0, 1024), (1024, 1024), (2048, 952)]

_CACHE = {}


def build_nc():
    import concourse.mybir as mybir
    from concourse import bacc
    from concourse.tile import TileContext

    f32 = mybir.dt.float32
    f32r = mybir.dt.float32r
    bf16 = mybir.dt.bfloat16
    u32 = mybir.dt.uint32
    Alu = mybir.AluOpType
    Act = mybir.ActivationFunctionType
    AX = mybir.AxisListType

    nc = bacc.Bacc(num_devices=NCORES)

    # natural-layout augmented x, pre-chunked host-side to [128, 24*65]
    xnm = nc.declare_dram_parameter("xnm", [128, NCHK * (DIN + 1)], f32, isOutput=False)
    xno = nc.declare_dram_parameter("xno", [128, NCHK * (DIN + 1)], f32, isOutput=False)
    # transposed augmented x for this core's row block
    xtr = nc.declare_dram_parameter("xtr", [DIN + 1, R], f32, isOutput=False)
    # packed weights: cols 0:65 Wa, 66:131 Wa^T (rows 0:64, col 131 zero
    # pad so f32r matmul free dims stay even), 133 gamma, 134 beta
    wgb = nc.declare_dram_parameter("wgb", [DIN + 1, 136], f32, isOutput=False)
    m0_p = nc.declare_dram_parameter("m0b", [1, 3008], bf16, isOutput=False)
    m1_p = nc.declare_dram_parameter("m1b", [1, 3008], bf16, isOutput=False)
    id_p = nc.declare_dram_parameter("ident", [128, 128], f32, isOutput=False)
    out_p = nc.declare_dram_parameter("out", [R, K, N], bf16, isOutput=True)

    with TileContext(nc, num_cores=NCORES) as tc:
        with tc.tile_pool(name="const", bufs=1) as cp:
            xn_m = cp.tile([128, NCHK, DIN + 1], f32)
            xn_o = cp.tile([128, NCHK, DIN + 1], f32)
            xtr_sb = cp.tile([DIN + 1, R], f32)
            wgb_sb = cp.tile([DIN + 1, 136], f32)
            ident = cp.tile([128, 128], f32)
            wat_r = cp.tile([D, DIN + 2], f32r)
            xt_sb = cp.tile([DIN + 1, NP], f32r)
            m0b = cp.tile([1, 3008], bf16)
            m1b = cp.tile([1, 3008], bf16)
            mb0 = cp.tile([128, 3008], bf16)
            mb1 = cp.tile([128, 3008], bf16)
            gs_m = cp.tile([D, D + 1], f32r)
            gs_o = cp.tile([D, D + 1], f32r)
            ert_sb = cp.tile([D, R], f32r)
            q_sb = cp.tile([D, R], f32r)
            gpqc = cp.tile([D, 768], f32r)
            w3t_sb = cp.tile([D, D + 2], f32r)
            v2_sb = cp.tile([DIN + 1, 832], f32r)
            sm = cp.tile([128, 16], f32)
            sq = cp.tile([D, 2 * D], f32)
            warm = cp.tile([DIN + 1, 512], bf16)

            # ---- loads: spread issue over SP / Act / Pool queues so the
            # critical xn transfers hit the DMA engines early ----
            HC = (NCHK // 2) * (DIN + 1)
            nc.sync.dma_start(
                out=xn_m[:, : NCHK // 2, :].rearrange("p c d -> p (c d)"),
                in_=xnm[:, 0:HC],
            )
            nc.sync.dma_start(out=ident[:, :], in_=id_p[:, :])
            nc.sync.dma_start(
                out=xn_m[:, NCHK // 2 :, :].rearrange("p c d -> p (c d)"),
                in_=xnm[:, HC:],
            )
            nc.sync.dma_start(out=xtr_sb[:, :], in_=xtr[:, :])
            nc.sync.dma_start(out=wgb_sb[:, :], in_=wgb[:, :])
            nc.scalar.dma_start(
                out=xn_o[:, : NCHK // 2, :].rearrange("p c d -> p (c d)"),
                in_=xno[:, 0:HC],
            )
            nc.scalar.dma_start(
                out=xn_o[:, NCHK // 2 :, :].rearrange("p c d -> p (c d)"),
                in_=xno[:, HC:],
            )
            nc.sync.dma_start(out=m0b[:, :], in_=m0_p[:, :])
            nc.sync.dma_start(out=m1b[:, :], in_=m1_p[:, :])

            # small const prep
            nc.vector.memset(warm[:, :], 0.0)
            nc.vector.memset(gpqc[:, :].bitcast(u32), 0)
            nc.vector.memset(v2_sb[:, :].bitcast(u32), 0)
            nc.vector.memset(sm[:, 11:12].bitcast(u32), 0x5F3759DF)
            nc.vector.memset(sm[:, 14:15].bitcast(u32), 0)
            nc.scalar.activation(sm[0:1, 15:16], sm[0:1, 14:15], Act.Exp)
            nc.vector.tensor_copy(wat_r[:, :], wgb_sb[0:D, 66:132])


            nc.gpsimd.partition_broadcast(mb0[:, :], m0b[:, :])
            nc.gpsimd.partition_broadcast(mb1[:, :], m1b[:, :])

            g_col = wgb_sb[0:D, 133:134]
            bt_col = wgb_sb[0:D, 134:135]

            # ---- pool A: warm PE, G both samples (interleaved halves),
            # transposes, per-sample stats, gs_m-dependent prep (ert/Q/ET)
            with tc.tile_pool(name="psA", bufs=1, space="PSUM") as pa:
                warm_ps = pa.tile([DIN + 1, 512], f32, tag="xtg", name="warmps", bufs=2)

                def warm_pe(n):
                    for _ in range(n):
                        nc.tensor.matmul(
                            warm_ps[:, :], lhsT=warm[:, 0 : DIN + 1],
                            rhs=warm[:, :], start=True, stop=True,
                        )

                warm_pe(4)  # span just the first xn transfer's latency

                xtg = [None] * 6
                xxp = {}

                def g_half(xsrc, tagc, h, transpose):
                    if tagc not in xxp:
                        xxp[tagc] = pa.tile(
                            [DIN + 1, DIN + 1], f32, tag="xx", name=f"xx{tagc}", bufs=3
                        )
                    for c in range(12 * h, 12 * h + 12):
                        nc.tensor.matmul(
                            xxp[tagc][:, :], lhsT=xsrc[:, c, :], rhs=xsrc[:, c, :],
                            start=(c == 0), stop=(c == NCHK - 1),
                        )
                        if transpose:
                            g = c // 4
                            if xtg[g] is None:
                                xtg[g] = pa.tile(
                                    [DIN + 1, 512], f32, tag="xtg", name=f"xtg{g}", bufs=2
                                )
                            nc.tensor.transpose(
                                xtg[g][:, 128 * (c % 4) : 128 * (c % 4 + 1)],
                                xsrc[:, c, :],
                                ident[:, :],
                            )
                            if c % 4 == 3:
                                if g % 2 == 0:
                                    nc.vector.tensor_copy(
                                        xt_sb[:, 512 * g : 512 * (g + 1)], xtg[g][:, :]
                                    )
                                else:
                                    nc.scalar.copy(
                                        xt_sb[:, 512 * g : 512 * (g + 1)], xtg[g][:, :]
                                    )

                def g_tail(gdst, tagc):
                    xx_sb = cp.tile([DIN + 1, DIN + 1], f32, name=f"xxsb{tagc}")
                    nc.vector.tensor_copy(xx_sb[:, :], xxp[tagc][:, :])
                    s2p = pa.tile([DIN + 1, DIN + 1], f32, tag="xx", name=f"s2{tagc}", bufs=3)
                    nc.tensor.matmul(
                        s2p[:, :], lhsT=xx_sb[:, :], rhs=wgb_sb[:, 0 : DIN + 1],
                        start=True, stop=True,
                    )
                    s2_sb = cp.tile([DIN + 1, DIN + 1], f32, name=f"s2sb{tagc}")
                    nc.vector.tensor_copy(s2_sb[:, :], s2p[:, :])
                    gap = pa.tile([DIN + 1, DIN + 1], f32, tag="xx", name=f"ga{tagc}", bufs=3)
                    nc.tensor.matmul(
                        gap[:, :], lhsT=wgb_sb[:, 0 : DIN + 1], rhs=s2_sb[:, :],
                        start=True, stop=True,
                    )
                    nc.vector.tensor_scalar_mul(gdst[:, :], gap[0:D, 0 : D + 1], 0.125)

                def sample_stats(gsx, smp):
                    # q8 = Gs^T Gs in cols 0:64, s1 (colsum(E)/8) in cols 64:66
                    qs1 = pa.tile([DIN + 1, 68], f32, tag="q8s", name=f"q8s{smp}", bufs=1)
                    nc.tensor.matmul(
                        qs1[0:D, 0:D], lhsT=gsx[:, 0:D], rhs=gsx[:, 0:D],
                        start=True, stop=True,
                    )
                    nc.tensor.matmul(
                        qs1[0:D, D : D + 2], lhsT=gsx[:, 0:D], rhs=gsx[:, D - 1 : D + 1],
                        start=True, stop=True,
                    )
                    nc.vector.tensor_tensor(
                        sq[:, D * smp : D * (smp + 1)], qs1[0:D, 0:D], gsx[:, 0:D], Alu.mult
                    )
                    nc.vector.reduce_sum(
                        sm[0:D, smp : smp + 1], sq[:, D * smp : D * (smp + 1)], axis=AX.X
                    )
                    nc.vector.tensor_copy(sm[0:D, 2 + smp : 3 + smp], qs1[0:D, D + 1 : D + 2])

                g_half(xn_m, "m", 0, True)
                g_half(xn_o, "o", 0, False)
                g_half(xn_m, "m", 1, True)
                g_half(xn_o, "o", 1, False)
                g_tail(gs_m, "m")
                sample_stats(gs_m, 0)
                # ert = Ein_r^T (needs only wgb/xtr); its Act copy overlaps
                # the other sample's tail below
                eqp = pa.tile([D, 768], f32, tag="eq", name="ertps", bufs=1)
                for c0, c1 in ((0, 512), (512, R)):
                    nc.tensor.matmul(
                        eqp[:, c0:c1], lhsT=wgb_sb[:, 0:D], rhs=xtr_sb[:, c0:c1],
                        start=True, stop=True,
                    )
                nc.scalar.copy(ert_sb[:, :], eqp[:, 0:R])
                g_tail(gs_o, "o")
                sample_stats(gs_o, 1)
                # Q = Gs ert;  W3T = Gs Wa^T (v2 lhsT, absorbs the Gs)
                qp = pa.tile([D, 768], f32, tag="eq", name="qps", bufs=1)
                for c0, c1 in ((0, 512), (512, R)):
                    nc.tensor.matmul(
                        qp[:, c0:c1], lhsT=gs_m[:, 0:D], rhs=ert_sb[:, c0:c1],
                        start=True, stop=True,
                    )
                nc.scalar.copy(q_sb[:, :], qp[:, 0:R])
                w3p = pa.tile([DIN + 1, 68], f32, tag="q8s", name="w3ps", bufs=1)
                nc.tensor.matmul(
                    w3p[0:D, 0 : D + 2], lhsT=gs_m[:, 0:D], rhs=wat_r[:, :],
                    start=True, stop=True,
                )
                nc.vector.tensor_copy(w3t_sb[:, :], w3p[0:D, 0 : D + 2])

            # ---- stats combine (fused tensor_scalar ops, short chain) ----
            mean = sm[0:D, 4:5]
            ex2 = sm[0:D, 5:6]
            msq = sm[0:D, 6:7]
            vpe = sm[0:D, 7:8]
            rstd = sm[0:D, 8:9]
            gp = sm[0:D, 9:10]
            negc = sm[0:D, 10:11]
            magic = sm[0:D, 11:12]
            i2 = sm[0:D, 12:13]
            t1 = sm[0:D, 13:14]
            cnt8 = 8.0 / float(B * N)
            nc.vector.tensor_scalar(
                mean, sm[0:D, 2:3], sm[0:D, 3:4], cnt8, Alu.add, Alu.mult
            )
            nc.vector.tensor_scalar(
                ex2, sm[0:D, 0:1], sm[0:D, 1:2], cnt8, Alu.add, Alu.mult
            )
            nc.vector.tensor_tensor(msq, mean, mean, Alu.mult)
            nc.vector.tensor_scalar(
                vpe, ex2, msq, float(EPS), Alu.subtract, Alu.add
            )
            # rstd = (var+eps)^-0.5: fast-inverse-sqrt seed + 2 Newton
            # steps (DVE-only: Act Sqrt would thrash the act-func table)
            nc.vector.tensor_scalar(
                i2.bitcast(mybir.dt.int32), vpe.bitcast(mybir.dt.int32),
                1, None, Alu.arith_shift_right,
            )
            nc.vector.tensor_tensor(
                rstd.bitcast(mybir.dt.int32), magic.bitcast(mybir.dt.int32),
                i2.bitcast(mybir.dt.int32), Alu.subtract,
            )
            for _ in range(2):
                nc.vector.tensor_tensor(t1, vpe, rstd, Alu.mult)
                nc.vector.tensor_tensor(t1, t1, rstd, Alu.mult)
                nc.vector.tensor_scalar(t1, t1, -0.5, 1.5, Alu.mult, Alu.add)
                nc.vector.tensor_tensor(rstd, rstd, t1, Alu.mult)
            nc.vector.tensor_tensor(gp, g_col, rstd, Alu.mult)
            nc.vector.tensor_scalar(
                negc, gp, mean, bt_col, Alu.mult, Alu.subtract
            )
            # phase-5 lhsT: v2 = Wa64 (gp (x) Q + cneg (x) ones)
            nc.vector.tensor_scalar(
                gpqc[:, 0:R], q_sb[:, :], gp, negc, Alu.mult, Alu.subtract
            )
            with tc.tile_pool(name="psB", bufs=1, space="PSUM") as pb:
                v2ps = pb.tile([DIN + 1, 768], f32, name="v2ps")
                for c0, c1 in ((0, 512), (512, R)):
                    nc.tensor.matmul(
                        v2ps[:, c0:c1], lhsT=w3t_sb[:, 0 : DIN + 1], rhs=gpqc[:, c0:c1],
                        start=True, stop=True,
                    )
                nc.scalar.copy(v2_sb[:, 0:R], v2ps[:, 0:R])

            # ---- phase 5: logits, softmax, weighted bf16 outputs ----
            # 3 psum pieces (1024/1024/952) cover the 3000 cols + pad.
            # Per-piece local-max softmax: exp(piece) only waits its own row
            # max; the global correction e_p = exp(m_p - M) and 1/S fold
            # into per-piece q-multiplies (g_p), split Act/DVE.  o0 = q*m0
            # on DVE, o1 = q*m1 mostly on Pool; one k-interleaved DMA/chunk.
            OSP = 2880   # Pool's share of the o1 weighting
            W08 = 3008
            PIECES5 = ((0, 1024), (1024, 1024), (2048, 960))
            with (
                tc.tile_pool(name="psL", bufs=1, space="PSUM") as pl,
                tc.tile_pool(name="pex", bufs=5) as pex,
                tc.tile_pool(name="pq", bufs=5) as pq,
                tc.tile_pool(name="pout", bufs=4) as pout,
                tc.tile_pool(name="pnm", bufs=4) as pnm,
            ):
                st = [dict() for _ in range(NCH)]

                def s1(c):
                    lgs = []
                    for p, (base, w) in enumerate(PIECES5):
                        lg = pl.tile([128, 1024], f32, tag="lg", name=f"lg{c}_{p}", bufs=4)
                        for c0, c1 in ((0, 512), (512, w)):
                            nc.tensor.matmul(
                                lg[:, c0:c1],
                                lhsT=v2_sb[:, IC * c : IC * c + 128],
                                rhs=xt_sb[:, base + c0 : base + c1],
                                start=True, stop=True,
                            )
                        lgs.append(lg)
                    st[c]["lg"] = lgs

                def s2(c):
                    # nm cols: 0:3 -localmax_p, 3 negM (clamped), 4:7 S_p,
                    # 7 S, 8 invS, 9:12 e_p -> g_p
                    nm = pnm.tile([128, 16], f32, tag="nm", name=f"nm{c}")
                    lgs = st[c]["lg"]
                    for p, (base, w) in enumerate(PIECES5):
                        nc.vector.reduce_max(
                            nm[:, p : p + 1], lgs[p][:, 0:w], axis=AX.X, negate=True
                        )
                    nc.vector.tensor_reduce(
                        nm[:, 3:4], nm[:, 0:3], axis=AX.X, op=Alu.min
                    )
                    nc.vector.tensor_scalar_min(nm[:, 3:4], nm[:, 3:4], 0.0)
                    st[c]["nm"] = nm

                def s3(c):
                    nm = st[c]["nm"]
                    lgs = st[c]["lg"]
                    pexp = pex.tile([128, W08], bf16, tag="pex", name=f"pex{c}")
                    for p, (base, w) in enumerate(PIECES5):
                        we = min(w, W08 - base)
                        nc.scalar.activation(
                            pexp[:, base : base + we],
                            lgs[p][:, 0:we],
                            Act.Exp,
                            bias=nm[:, p : p + 1],
                            accum_out=nm[:, 4 + p : 5 + p],
                        )
                    nc.scalar.activation(
                        nm[:, 9:12], nm[:, 0:3], Act.Exp,
                        bias=nm[:, 3:4], scale=-1.0,
                    )
                    nc.vector.tensor_tensor(
                        nm[:, 4:7], nm[:, 4:7], nm[:, 9:12], Alu.mult
                    )
                    nc.vector.reduce_sum(nm[:, 7:8], nm[:, 4:7], axis=AX.X)
                    nc.vector.reciprocal(nm[:, 8:9], nm[:, 7:8])
                    nc.vector.tensor_scalar(
                        nm[:, 9:12], nm[:, 9:12], nm[:, 8:9], None, Alu.mult
                    )
                    st[c]["p"] = pexp

                def s4(c):
                    r0 = IC * c
                    nm = st[c]["nm"]
                    pexp = st[c]["p"]
                    last = c == NCH - 1
                    q = pq.tile([128, W08], bf16, tag="q", name=f"q{c}")
                    if c == 0 or last:
                        nc.vector.tensor_scalar(
                            q[:, 0:1024], pexp[:, 0:1024], nm[:, 9:10], None, Alu.mult
                        )
                        nc.vector.tensor_scalar(
                            q[:, 1024:2048], pexp[:, 1024:2048], nm[:, 10:11],
                            None, Alu.mult,
                        )
                    else:
                        nc.scalar.mul(q[:, 0:1024], pexp[:, 0:1024], nm[:, 9:10])
                        nc.scalar.mul(q[:, 1024:2048], pexp[:, 1024:2048], nm[:, 10:11])
                    nc.vector.tensor_scalar(
                        q[:, 2048:W08], pexp[:, 2048:W08], nm[:, 11:12], None, Alu.mult
                    )
                    o01 = pout.tile([128, 2, W08], bf16, tag="o", name=f"o{c}")
                    osp = 1600 if last else OSP
                    nc.gpsimd.tensor_tensor(
                        o01[:, 1, 0:osp], q[:, 0:osp], mb1[:, 0:osp], Alu.mult
                    )
                    nc.vector.tensor_tensor(
                        o01[:, 0, :], q[:, :], mb0[:, :], Alu.mult
                    )
                    if last:
                        # drain: let the o0 half leave while Pool finishes o1
                        nc.sync.dma_start(
                            out=out_p[r0 : r0 + IC, 0:1, :],
                            in_=o01[0:IC, 0:1, 0:N],
                        )
                    nc.vector.tensor_tensor(
                        o01[:, 1, osp:W08], q[:, osp:W08], mb1[:, osp:W08], Alu.mult
                    )
                    if last:
                        nc.sync.dma_start(
                            out=out_p[r0 : r0 + IC, 1:2, :],
                            in_=o01[0:IC, 1:2, 0:N],
                        )
                    else:
                        nc.sync.dma_start(
                            out=out_p[r0 : r0 + IC, :, :], in_=o01[0:IC, :, 0:N]
                        )

                stages = (s1, s2, s3, s4)
                for step in range(NCH + 3):
                    for k, fn in enumerate(stages):
                        c = step - k
                        if 0 <= c < NCH:
                            fn(c)

    nc.compile()
    return nc


def make_in_maps(inputs):
    inp = np.asarray(inputs["input"], dtype=np.float32)
    m = np.asarray(inputs["m"], dtype=np.float32)
    W = np.asarray(inputs["W_in1"], dtype=np.float32)
    b1 = np.asarray(inputs["b_in1"], dtype=np.float32)
    g = np.asarray(inputs["bn2_gamma"], dtype=np.float32)
    bt = np.asarray(inputs["bn2_beta"], dtype=np.float32)

    wa = np.zeros((DIN + 1, D + 1), dtype=np.float32)
    wa[:DIN, :D] = W.T
    wa[DIN, :D] = b1
    wa[DIN, D] = 1.0  # unit column: passes the x ones-row through
    wgb = np.zeros((DIN + 1, 136), dtype=np.float32)
    wgb[:, 0 : D + 1] = wa
    wgb[0:D, 66:131] = wa.T[:D, :]
    wgb[0:D, 133] = g
    wgb[0:D, 134] = bt
    wgb = np.ascontiguousarray(wgb)
    import ml_dtypes
    bf = ml_dtypes.bfloat16
    mpad = np.zeros((K, 3008), dtype=np.float32)
    mpad[:, :N] = m
    m0b = np.ascontiguousarray(mpad[0:1, :].astype(bf))
    m1b = np.ascontiguousarray(mpad[1:2, :].astype(bf))
    ident = np.ascontiguousarray(np.eye(128, dtype=np.float32))

    xts = []
    xns = []
    for b in range(B):
        x = np.zeros((DIN + 1, NP), dtype=np.float32)
        x[:DIN, :N] = inp[b].T
        x[DIN, :N] = 1.0  # ones row (zero on the j-padding)
        xts.append(x)
        # natural layout, pre-chunked to [128, 24*(D+1)] for straight DMA
        xn = np.ascontiguousarray(
            x.T.reshape(NP // 128, 128, DIN + 1)
            .transpose(1, 0, 2)
            .reshape(128, (NP // 128) * (DIN + 1))
        )
        xns.append(xn)

    in_maps = []
    for c in range(NCORES):
        b, r = divmod(c, 4)
        in_maps.append(
            {
                "xnm": xns[b],
                "xno": xns[1 - b],
                "xtr": np.ascontiguousarray(xts[b][:, R * r : R * (r + 1)]),
                "wgb": wgb,
                "m0b": m0b,
                "m1b": m1b,
                "ident": ident,
            }
        )
    return in_maps


def kernel(**inputs):
    from concourse.bass_utils import run_bass_kernel_spmd

    if "nc" not in _CACHE:
        _CACHE["nc"] = build_nc()
    nc = _CACHE["nc"]
    in_maps = make_in_maps(inputs)
    res = run_bass_kernel_spmd(nc, in_maps, core_ids=list(range(NCORES))).results

    out = np.empty((K, B, N, N), dtype=np.float32)
    for c in range(NCORES):
        b, r = divmod(c, 4)
        out[:, b, R * r : R * (r + 1), :] = (
            np.asarray(res[c]["out"]).astype(np.float32).transpose(1, 0, 2)
        )
    return out
